# revision 1
# baseline (speedup 1.0000x reference)
"""Dynamic Directional Attention on 8 trn2 NeuronCores (Bass/Tile), v2.

Problem: B=4, L=S=2048, H=8, E=64, f32.
  qt = tanh(q * 1/std_H(q) * dw) * dyn   (std over H=8, ddof=1; eps dropped —
                                          it only matters when std < 1e-4)
  kt likewise; scores[b,h,l,s] = qt . kt  (contract E)
  tau[l] = sqrt(var_s(scores[l,:], ddof=1) + eps)
  A = softmax(scale * scores / tau);  out = A @ v  [B,L,H,E]

Sharding: 8 cores = 4 batches x 2 L-halves; per core q[1024, 512] plus full
k/v[2048, 512] (replicated across the half-pair). No collectives.

v2 design vs v1 (376us):
  - fp8e4 DoubleRow matmuls for both scores^T (K=2x32 over E) and A@V
    (K=2x128 over s-chunk pairs) -> PE cycles cut ~2x and ~4x.
  - "G-layout" grouped transposes: per group g (heads 4g..4g+3), stream A
    (e 0-31) and B (e 32-63) are transposed separately so head h's 64 e-rows
    land as [32 partitions x 2 blocks] -- exactly the DoubleRow operand
    layout. Same transpose/copy count as v1.
  - Score row-stats via block-diagonal Gram in fp8 G-layout (ws = G~ @ tq,
    rs = ksum~ @ tq as DR matmuls), tau via ln/exp (same ACT table set as
    the main exp): 3 activation-table loads total.
  - A@V: vb stationary [128,2,96] (64 v-cols + ones + pad to the 32-multiple
    ISA requirement), at moving; accumulates out^T[d,l] in PSUM; PE f32
    transpose back, then per-partition reciprocal scaling.
  - exp: ACT from PSUM -> at fp8 directly; optional per-head tail of kk
    tiles computed on DVE via a Schraudolph-style bit-trick exp
    (x*a+b -> int8 -> bitcast fp8e4) to balance ACT vs DVE.
"""

import os
import sys

for _p in ("/opt/trn_rl_repo", "/root/.axon_site/_ro/trn_rl_repo"):
    if os.path.isdir(_p) and _p not in sys.path:
        sys.path.append(_p)

import numpy as np

import concourse.bass as bass
import concourse.mybir as mybir
import concourse.tile as tile
from concourse import bacc
from concourse.bass_utils import run_bass_kernel_spmd
from concourse.masks import make_identity

F32 = mybir.dt.float32
BF16 = mybir.dt.bfloat16
FP8 = mybir.dt.float8e4
I8 = mybir.dt.int8
I16 = mybir.dt.int16
AF = mybir.ActivationFunctionType
ALU = mybir.AluOpType
DR = mybir.MatmulPerfMode.DoubleRow

B, L, S, H, E = 4, 2048, 2048, 8, 64
LC = L // 2          # 1024 l-rows per core
D = H * E            # 512 free-dim columns (all 8 heads)
P = 128
NLT = LC // P        # 8 l-chunks (q)
NST = S // P         # 16 s-chunks (k, v)
EPS = 1e-6
SCALE = 1.0 / np.sqrt(E)
UNB_H = float(H) / float(H - 1)
UNB_S = float(S) / float(S - 1)

QSF = 64.0           # fp8 pre-scale folded into m; exp uses scale=1/QSF
LOG2E = 1.4426950408889634
SCH_C = 0.04303      # Schraudolph mean-centering
SCH_A = 128.0 * LOG2E / QSF          # bf16 target: exp bits = x*A + B (int16)
SCH_B = 128.0 * (127.0 - SCH_C)

# kk tiles per head exp'ed on DVE (Schraudolph) instead of ACT; tail of range
N_DVE_EXP = 0

_last_exec_time_ns = None


def _ensure_axon_hooks():
    """Provide antenv.axon_hooks (NTFF profiling hook) if the image lacks it."""
    try:
        import antenv.axon_hooks  # noqa: F401

        return
    except ImportError:
        pass
    import contextlib
    import ctypes
    import types

    try:
        import antenv
    except ImportError:
        return

    holder = {"h": None}
    mod = types.ModuleType("antenv.axon_hooks")
    mod.set_axon_ntff_profile_hook = lambda h: holder.__setitem__("h", h)
    mod.get_axon_ntff_profile_hook = lambda: holder["h"]
    sys.modules["antenv.axon_hooks"] = mod
    antenv.axon_hooks = mod

    so_path = "/opt/axon/libaxon_pjrt.so"
    if not os.path.exists(so_path):
        return
    try:
        lib = ctypes.CDLL(so_path)
    except OSError:
        return
    if not hasattr(lib, "axon_start_nrt_profile"):
        return
    lib.axon_start_nrt_profile.argtypes = [
        ctypes.POINTER(ctypes.c_int64),
        ctypes.c_size_t,
    ]
    lib.axon_start_nrt_profile.restype = ctypes.c_int64
    lib.axon_stop_nrt_profile.argtypes = [ctypes.c_char_p]
    lib.axon_stop_nrt_profile.restype = ctypes.c_int64

    @contextlib.contextmanager
    def _hook(output_dir, device_ids):
        import jax

        jax.devices()
        if device_ids:
            ids = (ctypes.c_int64 * len(device_ids))(*device_ids)
            rc = lib.axon_start_nrt_profile(ids, len(device_ids))
        else:
            rc = lib.axon_start_nrt_profile(None, 0)
        if rc != 0:
            raise RuntimeError(f"axon_start_nrt_profile rc={rc}")
        try:
            yield
        finally:
            n = lib.axon_stop_nrt_profile(str(output_dir).encode())
            print(f"profile: {n} file(s) written to {output_dir}", file=sys.stderr)

    holder["h"] = _hook


def _head_bcast(ap_2d, nh=H, ne=E):
    """View a [p, ne] AP as [p, nh, ne] with the head dim broadcast (step 0)."""
    return bass.AP(
        tensor=ap_2d.tensor,
        offset=ap_2d.offset,
        ap=[list(ap_2d.ap[0]), [0, nh], list(ap_2d.ap[1])],
    )


def _blk_bcast(ap_2d, nblk=2):
    """View a [p, n] AP as [p, nblk, n] with the block dim broadcast."""
    return bass.AP(
        tensor=ap_2d.tensor,
        offset=ap_2d.offset,
        ap=[list(ap_2d.ap[0]), [0, nblk], list(ap_2d.ap[1])],
    )


def build_nc():
    nc = bacc.Bacc("TRN2", target_bir_lowering=False, debug=False)
    q_d = nc.dram_tensor("q", [LC, D], F32, kind="ExternalInput")
    k_d = nc.dram_tensor("k", [S, D], F32, kind="ExternalInput")
    v_d = nc.dram_tensor("v", [S, D], F32, kind="ExternalInput")
    dw_d = nc.dram_tensor("dw", [1, 1], F32, kind="ExternalInput")
    dp_d = nc.dram_tensor("dp", [1, 1], F32, kind="ExternalInput")
    o_d = nc.dram_tensor("o", [LC, D], F32, kind="ExternalOutput")

    q_r = q_d.rearrange("(n p) d -> p n d", p=P)
    k_r = k_d.rearrange("(n p) d -> p n d", p=P)
    v_r = v_d.rearrange("(n p) d -> p n d", p=P)
    o_r = o_d.rearrange("(n p) d -> p n d", p=P)

    from contextlib import ExitStack

    with tile.TileContext(nc) as tc, ExitStack() as ctx:
        ek = ctx.enter_context
        sing = ek(tc.tile_pool(name="sing", bufs=1))
        pnat = ek(tc.tile_pool(name="nat", bufs=3))      # rotating nat chunks
        psq = ek(tc.tile_pool(name="sq", bufs=2))        # squared scratch
        pvw = ek(tc.tile_pool(name="varw", bufs=1))      # var/rstd batches
        ptn = ek(tc.tile_pool(name="tn", bufs=4))        # transformed bf16
        pgl = ek(tc.tile_pool(name="glay", bufs=1))      # TK_G/TQ_G/QS_G fp8
        pgb = ek(tc.tile_pool(name="gbd", bufs=1))       # block-diag G, ksum
        ppr = ek(tc.tile_pool(name="prod", bufs=1))      # prod bf16
        prw = ek(tc.tile_pool(name="rows", bufs=1))      # [8,1024] stat rows
        pat = ek(tc.tile_pool(name="at", bufs=2))        # A^T fp8 per head
        pvb = ek(tc.tile_pool(name="vb", bufs=1))        # V + ones fp8
        pout = ek(tc.tile_pool(name="osb", bufs=1))      # out staging f32
        psc = ek(tc.tile_pool(name="small", bufs=4))
        pdr = ek(tc.tile_pool(name="dr", bufs=1, space="DRAM"))

        # --- constants ---
        ident = sing.tile([P, P], BF16)
        make_identity(nc, ident)
        dw_t = sing.tile([P, 1], F32)
        nc.sync.dma_start(out=dw_t, in_=dw_d[:, :].to_broadcast([P, 1]))
        dp_t = sing.tile([P, 1], F32)
        nc.sync.dma_start(out=dp_t, in_=dp_d[:, :].to_broadcast([P, 1]))
        dp2 = sing.tile([P, 1], F32)
        nc.vector.tensor_mul(dp2, dp_t, dp_t)
        dp4u = sing.tile([P, 1], F32)  # dyn^4 * UNB_S  (ln scale)
        nc.vector.tensor_mul(dp4u, dp2, dp2)
        nc.vector.tensor_scalar_mul(dp4u, dp4u, float(UNB_S))
        c2 = sing.tile([P, 1], F32)    # QSF * SCALE * dyn^2
        nc.vector.tensor_scalar_mul(c2, dp2, float(QSF * SCALE))
        eps_t = sing.tile([P, 1], F32)
        nc.vector.memset(eps_t, EPS)
        ones4 = sing.tile([P, 4], BF16)  # block-column ones for 32-row sums
        nc.vector.memset(ones4, 0.0)
        for r in range(4):
            nc.vector.memset(ones4[32 * r : 32 * (r + 1), r : r + 1], 1.0)

        # ================= PHASE 1: per-(l,e) stats (nat rotated) ============
        all_chunks = [("k", i, k_r) for i in range(NST)] + \
                     [("q", i, q_r) for i in range(NLT)]
        rstds = {}  # (kind, i) -> rstd AP [128, 64]
        for b0 in range(0, len(all_chunks), 4):
            batch = all_chunks[b0 : b0 + 4]
            ssum_b = pvw.tile([P, 4, E], F32, tag="ssum", bufs=2)
            ssq_b = pvw.tile([P, 4, E], F32, tag=f"ssq{b0}")
            for j, (kind, i, src) in enumerate(batch):
                nat = pnat.tile([P, D], F32, tag="nat", name=f"nat_{kind}{i}")
                nc.sync.dma_start(out=nat, in_=src[:, i, :])
                sq = psq.tile([P, D], F32, tag="sq")
                nc.gpsimd.tensor_mul(sq, nat, nat)
                nc.vector.tensor_reduce(
                    ssum_b[:, j, :], nat.rearrange("p (h e) -> p e h", h=H),
                    axis=mybir.AxisListType.X, op=ALU.add)
                nc.vector.tensor_reduce(
                    ssq_b[:, j, :], sq.rearrange("p (h e) -> p e h", h=H),
                    axis=mybir.AxisListType.X, op=ALU.add)
            # t = ssq - ssum^2/H ; std = sqrt((UNB_H/H) * t); rstd = 1/std
            nc.vector.tensor_mul(ssum_b, ssum_b, ssum_b)
            nc.vector.tensor_scalar_mul(ssum_b, ssum_b, 1.0 / H)
            nc.vector.tensor_sub(ssq_b, ssq_b, ssum_b)
            nc.scalar.activation(ssq_b, ssq_b, AF.Sqrt, bias=0.0,
                                 scale=float(UNB_H / H))
            nc.vector.reciprocal(ssq_b, ssq_b)
            for j, (kind, i, src) in enumerate(batch):
                rstds[(kind, i)] = ssq_b[:, j, :]

        # ============ PHASE 2: transform + transpose + gram =================
        # G-layout targets
        tk_g = [pgl.tile([P, 2, S], FP8, tag=f"tkg{g}", name=f"tkg{g}")
                for g in range(2)]
        tq_g = [pgl.tile([P, 2, LC], FP8, tag=f"tqg{g}", name=f"tqg{g}")
                for g in range(2)]
        tq_gb = [pgl.tile([P, 2, LC], BF16, tag=f"tqgb{g}", name=f"tqgb{g}")
                 for g in range(2)]

        with tc.tile_pool(name="pro_ps", bufs=2, space="PSUM") as ppro:
            # single multi-region accumulators in G-row layout
            g2_all = ppro.tile([32, 2, H, E], F32, tag="g2", bufs=1)
            ks_all = ppro.tile([32, 2, H], F32, tag="ks", bufs=1,
                               padded_shape=[32, 2, 256])
            ones1 = sing.tile([P, 1], BF16)
            nc.vector.memset(ones1, 1.0)

            def transform(kind, i, src, resident=False):
                # tanh written pre-swizzled: tg[p, g, blk, h_local, e_sub]
                nat = pnat.tile([P, D], F32, tag="nat", name=f"nat2_{kind}{i}")
                nc.sync.dma_start(out=nat, in_=src[:, i, :])
                tmp = psq.tile([P, D], F32, tag="tmp")
                nc.gpsimd.tensor_mul(tmp, nat, _head_bcast(rstds[(kind, i)]))
                tag = f"tgk{i}" if resident else "tg"
                tg = ptn.tile([P, 2, 2, 4, 32], BF16, tag=tag,
                              name=f"tg_{kind}{i}", bufs=1 if resident else None)
                for g in range(2):
                    nc.scalar.activation(
                        tg[:, g].rearrange("p b h e -> p h b e"),
                        tmp[:, 256 * g : 256 * (g + 1)],
                        AF.Tanh, bias=0.0, scale=dw_t)
                return tg

            def emit_gram_all(tg_list):
                # each PSUM region's 16-chunk chain runs consecutively:
                # concurrently-open chains sharing a 2KB bank corrupt each
                # other via the start-flag zero-region
                for h in range(H):
                    g, hl = h // 4, h % 4
                    for bi in range(2):
                        for bj in range(2):
                            for i, tg in enumerate(tg_list):
                                nc.tensor.matmul(
                                    g2_all[:, bi, h, 32 * bj : 32 * (bj + 1)],
                                    tg[:, g, bi, hl, :], tg[:, g, bj, hl, :],
                                    start=(i == 0), stop=(i == NST - 1))
                        for i, tg in enumerate(tg_list):
                            nc.tensor.matmul(
                                ks_all[:, bi, h : h + 1], tg[:, g, bi, hl, :],
                                ones1, start=(i == 0), stop=(i == NST - 1))

            def emit_transposes(tgs, i0, dst, ncols, dstb=None):
                # tgs: list of 2 chunks (i0, i0+1); dst: tk_g or tq_g (fp8);
                # dstb: optional extra bf16 copy (stats path)
                for g in range(2):
                    for blk in range(2):
                        pt = ppro.tile([P, 2, P], BF16, tag="pt",
                                       padded_shape=[P, 2, 512])
                        for u, tg in enumerate(tgs):
                            nc.tensor.transpose(pt[:, u, :], tg[:, g, blk],
                                                ident)
                        dst_ap = dst[g][:, blk, i0 * P : (i0 + 2) * P].rearrange(
                            "p (a b) -> p a b", a=2)
                        if blk == 0:
                            nc.vector.tensor_copy(dst_ap, pt)
                        else:
                            nc.scalar.copy(dst_ap, pt)
                        if dstb is not None:
                            nc.vector.tensor_copy(
                                dstb[g][:, blk,
                                        i0 * P : (i0 + 2) * P].rearrange(
                                    "p (a b) -> p a b", a=2),
                                pt)

            tns_pend = []
            tgk_list = []
            for i in range(NST):
                tn = transform("k", i, k_r, resident=True)
                tgk_list.append(tn)
                tns_pend.append(tn)
                if len(tns_pend) == 2:
                    emit_transposes(tns_pend, i - 1, tk_g, S)
                    tns_pend = []
            emit_gram_all(tgk_list)
            for i in range(NLT):
                tn = transform("q", i, q_r)
                tns_pend.append(tn)
                if len(tns_pend) == 2:
                    emit_transposes(tns_pend, i - 1, tq_g, LC, dstb=tq_gb)
                    tns_pend = []

            # block-diag G (fp8, scaled 1/S) and ksum (scaled 1/64)
            gbd = [[pgb.tile([P, 2, P], BF16, tag=f"gbd{g}{j}", name=f"gbd{g}{j}")
                    for j in range(2)] for g in range(2)]
            ksbd = [pgb.tile([P, 2, 32], BF16, tag=f"ksbd{g}", name=f"ksbd{g}")
                    for g in range(2)]
            for g in range(2):
                for j in range(2):
                    nc.vector.memset(gbd[g][j], 0.0)
                nc.vector.memset(ksbd[g], 0.0)
            for h in range(H):
                g, r = h // 4, h % 4
                for j in range(2):
                    nc.vector.tensor_scalar_mul(
                        gbd[g][j][32 * r : 32 * (r + 1), :, 32 * r : 32 * (r + 1)],
                        g2_all[:, :, h, 32 * j : 32 * (j + 1)], 1.0 / S)
                for i in range(2):
                    nc.vector.tensor_scalar_mul(
                        ksbd[g][32 * r : 32 * (r + 1), i, r : r + 1],
                        ks_all[:, i, h : h + 1], 1.0 / 64.0)

        # ================= PHASE 3: score row stats -> m ====================
        ssq_sb = prw.tile([4, 2, LC], BF16, tag="ssqsb")
        rsum_sb = prw.tile([4, 2, LC], BF16, tag="rsumsb")
        with tc.tile_pool(name="st_ps", bufs=1, space="PSUM") as pst:
            for g in range(2):
                ws_b = [pst.tile([P, LC], F32, tag="ws", name=f"ws{g}{j}",
                                 bufs=2) for j in range(2)]
                for j in range(2):
                    for n0 in range(0, LC, 256):
                        for blk in range(2):
                            nc.tensor.matmul(
                                ws_b[j][:, n0 : n0 + 256],
                                gbd[g][j][:, blk, :],
                                tq_gb[g][:, blk, n0 : n0 + 256],
                                start=(blk == 0), stop=(blk == 1))
                prod = ppr.tile([P, 2, LC], BF16, tag="prod")
                for j in range(2):
                    nc.vector.tensor_mul(prod[:, j, :], tq_gb[g][:, j, :],
                                         ws_b[j])
                rs_ps = pst.tile([32, LC], F32, tag="rs", bufs=1)
                for n0 in range(0, LC, 256):
                    for blk in range(2):
                        nc.tensor.matmul(rs_ps[:, n0 : n0 + 256],
                                         ksbd[g][:, blk, :],
                                         tq_gb[g][:, blk, n0 : n0 + 256],
                                         start=(blk == 0), stop=(blk == 1))
                ssq_ps = pst.tile([4, LC], F32, tag="ssqp", bufs=1)
                for half in range(2):
                    for blk in range(2):
                        nc.tensor.matmul(
                            ssq_ps[:, 512 * half : 512 * (half + 1)],
                            ones4, prod[:, blk, 512 * half : 512 * (half + 1)],
                            start=(blk == 0), stop=(blk == 1))
                nc.vector.tensor_copy(ssq_sb[:, g, :], ssq_ps)
                nc.vector.tensor_copy(rsum_sb[:, g, :], rs_ps[0:4, :])

        # m = c2 / tau', tau'^2 = dyn^4*UNB*(ssq - rsum^2/1024) + eps
        # via ln/exp (same table set as main exp)
        nc.vector.tensor_mul(rsum_sb, rsum_sb, rsum_sb)
        nc.vector.tensor_scalar_mul(rsum_sb, rsum_sb, 1.0 / 1024.0)
        nc.vector.tensor_sub(ssq_sb, ssq_sb, rsum_sb)
        ln_c2 = sing.tile([4, 1], F32)
        nc.scalar.activation(ln_c2, c2[0:4, :], AF.Ln, bias=0.0, scale=1.0)
        nc.scalar.activation(ssq_sb, ssq_sb, AF.Ln, bias=eps_t[0:4, :],
                             scale=dp4u[0:4, :])
        nc.scalar.activation(ssq_sb, ssq_sb, AF.Exp, bias=ln_c2, scale=-0.5)
        mdr = pdr.tile([4, 2, LC], BF16, tag="mdr")
        nc.sync.dma_start(out=mdr[:, :, :], in_=ssq_sb)

        # qs = tq * m (broadcast m rows per head across its 32 partitions)
        qs_g = []
        for g in range(2):
            mb = psc.tile([P, LC], BF16, tag="mb", bufs=2)
            for r in range(4):
                nc.sync.dma_start(out=mb[32 * r : 32 * (r + 1), :],
                                  in_=mdr[r : r + 1, g, :].to_broadcast([32, LC]))
            qs = pgl.tile([P, 2, LC], FP8, tag=f"qsg{g}", name=f"qsg{g}")
            nc.vector.tensor_mul(qs, tq_g[g], _blk_bcast(mb))
            qs_g.append(qs)

        # head r=3 of each group sits at base partition 96, which matmul
        # operands can't address (base must be 0/32/64) — bounce to base 0.
        tk3_g, qs3_g = [], []
        for g in range(2):
            tk3 = pgl.tile([32, 2, S], FP8, tag=f"tk3{g}", name=f"tk3_{g}")
            nc.sync.dma_start(out=tk3, in_=tk_g[g][96:128, :, :])
            tk3_g.append(tk3)
            qs3 = pgl.tile([32, 2, LC], FP8, tag=f"qs3{g}", name=f"qs3_{g}")
            nc.sync.dma_start(out=qs3, in_=qs_g[g][96:128, :, :])
            qs3_g.append(qs3)

        # ================= PHASE 4: V + ones (fp8) ==========================
        # va[p, kk, h, 66]: 64 v-cols + ones col + pad, bf16
        va = pvb.tile([P, NST, H, 66], BF16, tag="va")
        for kk in range(NST):
            vn = pnat.tile([P, D], F32, tag="nat", name=f"vn{kk}")
            nc.sync.dma_start(out=vn, in_=v_r[:, kk, :])
            nc.gpsimd.tensor_copy(
                va[:, kk, :, 0:E],
                vn.rearrange("p (h e) -> p h e", h=H))
        nc.vector.memset(va.rearrange("p n h c -> p (n h) c")[:, :, E : E + 1],
                         1.0)

        # ================= PHASE 5: main loop ===============================

        with tc.tile_pool(name="mm_ps", bufs=2, space="PSUM") as pmm:

            def emit_exp(h, at, st_ps, kk):
                if kk >= NST - N_DVE_EXP:
                    nc.vector.tensor_scalar(
                        out=at[:, kk, :].bitcast(I16), in0=st_ps,
                        scalar1=SCH_A, scalar2=SCH_B,
                        op0=ALU.mult, op1=ALU.add)
                else:
                    nc.scalar.activation(at[:, kk, :], st_ps, AF.Exp,
                                         bias=0.0, scale=1.0 / QSF)

            def emit_av_lt(h, at, lt):
                # one l-tile's 16-step bf16 chain: po[128, 65] += at^T @ va
                po = av_state[h]["po"]
                if lt == 0:
                    po[0] = pmm.tile([P, NLT, E + 1], F32, tag="pob", bufs=2,
                                     name=f"pob{h}",
                                     padded_shape=[P, NLT, 128])
                for kx in range(NST):
                    nc.tensor.matmul(
                        po[0][:, lt, :],
                        at[:, kx, lt * P : (lt + 1) * P],
                        va[:, kx, h, 0 : E + 1],
                        start=(kx == 0), stop=(kx == NST - 1))

            def emit_epilogue(h):
                po = av_state[h]["po"][0]
                rc = psc.tile([P, NLT], F32, tag="rc")
                nc.vector.reciprocal(rc, po[:, :, E])
                for lt in range(NLT):
                    ob = psc.tile([P, 1, E], F32, tag="ob")
                    nc.vector.tensor_scalar_mul(
                        ob[:, 0, :], po[:, lt, 0:E], rc[:, lt : lt + 1])
                    nc.sync.dma_start(
                        out=o_r[:, lt : lt + 1, h * E : (h + 1) * E], in_=ob)

            av_state = {h: {"po": {}} for h in range(H)}
            prev = None
            for h in range(H):
                # emit st+exp for h interleaved with AV for h-1
                g, r = h // 4, h % 4
                if r == 3:
                    tks, qss = tk3_g[g], qs3_g[g]
                else:
                    tks = tk_g[g][32 * r : 32 * (r + 1)]
                    qss = qs_g[g][32 * r : 32 * (r + 1)]
                at = pat.tile([P, NST, LC], BF16, tag="at")
                for kk in range(NST):
                    st_ps = pmm.tile([P, LC], F32, tag="stp", bufs=2)
                    for n0 in range(0, LC, 256):
                        nc.tensor.matmul(
                            st_ps[:, n0 : n0 + 256],
                            tks[:, :, kk * P : (kk + 1) * P],
                            qss[:, :, n0 : n0 + 256],
                            start=True, stop=True, perf_mode=DR)
                    emit_exp(h, at, st_ps, kk)
                    if prev is not None and kk % 2 == 1:
                        ph, pat_t = prev
                        emit_av_lt(ph, pat_t, kk // 2)
                        if kk == NST - 1:
                            emit_epilogue(ph)
                prev = (h, at)
            # last head's AV + epilogue
            ph, pat_t = prev
            for lt in range(NLT):
                emit_av_lt(ph, pat_t, lt)
            emit_epilogue(ph)

    return nc


_nc_cache = None


def kernel(queries, keys, values, attn_mask=None, directional_weights=None,
           dynamic_param=None, **_unused):
    global _nc_cache, _last_exec_time_ns
    q = np.asarray(queries, dtype=np.float32)
    k = np.asarray(keys, dtype=np.float32)
    v = np.asarray(values, dtype=np.float32)
    if directional_weights is None:
        dw = np.ones((1, 1), dtype=np.float32)
    else:
        dw = np.asarray(directional_weights, dtype=np.float32).reshape(1, 1)
    if dynamic_param is None:
        dp = np.ones((1, 1), dtype=np.float32)
    else:
        dp = np.asarray(dynamic_param, dtype=np.float32).reshape(1, 1)

    if _nc_cache is None:
        nc = build_nc()
        nc.finalize()
        _nc_cache = nc
    nc = _nc_cache

    in_maps = []
    for c in range(8):
        b, lh = c // 2, c % 2
        in_maps.append({
            "q": np.ascontiguousarray(q[b, lh * LC : (lh + 1) * LC]).reshape(LC, D),
            "k": np.ascontiguousarray(k[b]).reshape(S, D),
            "v": np.ascontiguousarray(v[b]).reshape(S, D),
            "dw": dw, "dp": dp,
        })

    tracing = bool(os.environ.get("BASS_TRACE"))
    if tracing:
        _ensure_axon_hooks()
        import concourse.bass_utils as _bu

        _orig_upload = _bu.upload_artifacts
        _bu.upload_artifacts = lambda d: d
        try:
            res = run_bass_kernel_spmd(nc, in_maps, core_ids=list(range(8)))
        except Exception as e:
            print(f"traced run failed ({e!r}); retrying untraced", file=sys.stderr)
            os.environ["BASS_NEVER_TRACE"] = "1"
            try:
                res = run_bass_kernel_spmd(nc, in_maps, core_ids=list(range(8)))
            finally:
                os.environ.pop("BASS_NEVER_TRACE", None)
        finally:
            _bu.upload_artifacts = _orig_upload
    else:
        res = run_bass_kernel_spmd(nc, in_maps, core_ids=list(range(8)))
    _last_exec_time_ns = res.exec_time_ns

    out = np.empty((B, L, H, E), dtype=np.float32)
    for c in range(8):
        b, lh = c // 2, c % 2
        out[b, lh * LC : (lh + 1) * LC] = res.results[c]["o"].reshape(LC, H, E)
    return out



# revision 19
# speedup vs baseline: 1.1719x; 1.1719x over previous
"""Dynamic Directional Attention on 8 trn2 NeuronCores (Bass/Tile), v3.

Problem: B=4, L=S=2048, H=8, E=64, f32.
  qt = tanh(q * 1/std_H(q) * dw) * dyn   (std over H=8, ddof=1; eps dropped)
  kt likewise; scores[b,h,l,s] = qt . kt  (contract E)
  tau[l] = sqrt(var_s(scores[l,:], ddof=1) + eps)
  A = softmax(scale * scores / tau);  out = A @ v  [B,L,H,E]

Sharding: 8 cores = 4 batches x 2 L-halves; per core q[1024, 512] plus full
k/v[2048, 512]. No collectives.

v3 design vs v2 (384us):
  - scores matmuls WITHOUT DoubleRow: DR streams at ~2 cyc/row on HW while
    plain fp8/bf16 streams 1 cyc/row at 2.4GHz. K=64 fits the array anyway.
    Layout: head-pair stacking [p=(hi*64+e), g, s/l], stationary tk [64,128],
    moving qs [64,512] -> out [128 s, 512 l] in PSUM.
  - single-pass fused preamble: each chunk loaded ONCE; per-chunk
    bn_stats (mean/M2 over H in one DVE pass) + gpsimd even/odd-combine +
    1-step Newton rsqrt on DVE (no ACT Sqrt table load); tanh natural-layout
    [p, h, 65] with fused ones column for k; PE transposes into pair layout.
  - Gram per head [64,64]+ksum col in one chained matmul set; ws/ssq/rs via
    block-diag pair matmuls; m broadcast via PE selector matmul (no DRAM
    bounce). ACT tables: tanh set, then natural_log_exp for m + main exp.
  - A@V unchanged (bf16, at-stationary, ones column for the denominator).
"""

import os
import sys

for _p in ("/opt/trn_rl_repo", "/root/.axon_site/_ro/trn_rl_repo"):
    if os.path.isdir(_p) and _p not in sys.path:
        sys.path.append(_p)

import numpy as np

import concourse.bass as bass
import concourse.mybir as mybir
import concourse.tile as tile
from concourse import bacc
from concourse.bass_utils import run_bass_kernel_spmd
from concourse.masks import make_identity

F32 = mybir.dt.float32
BF16 = mybir.dt.bfloat16
FP8 = mybir.dt.float8e4
I8 = mybir.dt.int8
I16 = mybir.dt.int16
I32 = mybir.dt.int32
AF = mybir.ActivationFunctionType
ALU = mybir.AluOpType

B, L, S, H, E = 4, 2048, 2048, 8, 64
LC = L // 2          # 1024 l-rows per core
D = H * E            # 512 free-dim columns (all 8 heads)
P = 128
NLT = LC // P        # 8 l-chunks (q)
NST = S // P         # 16 s-chunks (k, v)
HG = H // 2          # 4 head-pair groups; head = 2g + hi
EPS = 1e-6
SCALE = 1.0 / np.sqrt(E)
SQ7 = float(np.sqrt(H - 1))      # rstd = sqrt(7) * rsqrt(M2), folded in tanh
UNB_S = float(S) / float(S - 1)

QSF = 64.0           # fp8 pre-scale folded into m; exp uses scale=1/QSF
LOG2E = 1.4426950408889634
SCH_C = 0.04303      # Schraudolph mean-centering
SCH_A = 128.0 * LOG2E / QSF          # bf16 target: exp bits = x*A + B (int16)
SCH_B = 128.0 * (127.0 - SCH_C)
MAGIC = 0x5F3759DF   # rsqrt seed

# exp tiles per head routed to DVE (Schraudolph) instead of ACT
N_DVE_EXP = 0

_last_exec_time_ns = None


def _ensure_axon_hooks():
    """Provide antenv.axon_hooks (NTFF profiling hook) if the image lacks it."""
    try:
        import antenv.axon_hooks  # noqa: F401

        return
    except ImportError:
        pass
    import contextlib
    import ctypes
    import types

    try:
        import antenv
    except ImportError:
        return

    holder = {"h": None}
    mod = types.ModuleType("antenv.axon_hooks")
    mod.set_axon_ntff_profile_hook = lambda h: holder.__setitem__("h", h)
    mod.get_axon_ntff_profile_hook = lambda: holder["h"]
    sys.modules["antenv.axon_hooks"] = mod
    antenv.axon_hooks = mod

    so_path = "/opt/axon/libaxon_pjrt.so"
    if not os.path.exists(so_path):
        return
    try:
        lib = ctypes.CDLL(so_path)
    except OSError:
        return
    if not hasattr(lib, "axon_start_nrt_profile"):
        return
    lib.axon_start_nrt_profile.argtypes = [
        ctypes.POINTER(ctypes.c_int64),
        ctypes.c_size_t,
    ]
    lib.axon_start_nrt_profile.restype = ctypes.c_int64
    lib.axon_stop_nrt_profile.argtypes = [ctypes.c_char_p]
    lib.axon_stop_nrt_profile.restype = ctypes.c_int64

    @contextlib.contextmanager
    def _hook(output_dir, device_ids):
        import jax

        jax.devices()
        if device_ids:
            ids = (ctypes.c_int64 * len(device_ids))(*device_ids)
            rc = lib.axon_start_nrt_profile(ids, len(device_ids))
        else:
            rc = lib.axon_start_nrt_profile(None, 0)
        if rc != 0:
            raise RuntimeError(f"axon_start_nrt_profile rc={rc}")
        try:
            yield
        finally:
            n = lib.axon_stop_nrt_profile(str(output_dir).encode())
            print(f"profile: {n} file(s) written to {output_dir}", file=sys.stderr)

    holder["h"] = _hook


def _hbcast(ap_2d, nh=H):
    """View a [p, ne] AP as [p, nh, ne] with the head dim broadcast (step 0)."""
    return bass.AP(
        tensor=ap_2d.tensor,
        offset=ap_2d.offset,
        ap=[list(ap_2d.ap[0]), [0, nh], list(ap_2d.ap[1])],
    )


def build_nc():
    nc = bacc.Bacc("TRN2", target_bir_lowering=False, debug=False)
    q_d = nc.dram_tensor("q", [LC, D], F32, kind="ExternalInput")
    k_d = nc.dram_tensor("k", [S, D], F32, kind="ExternalInput")
    v_d = nc.dram_tensor("v", [S, D], F32, kind="ExternalInput")
    dw_d = nc.dram_tensor("dw", [1, 1], F32, kind="ExternalInput")
    dp_d = nc.dram_tensor("dp", [1, 1], F32, kind="ExternalInput")
    o_d = nc.dram_tensor("o", [LC, D], F32, kind="ExternalOutput")

    q_r = q_d.rearrange("(n p) d -> p n d", p=P)
    k_r = k_d.rearrange("(n p) d -> p n d", p=P)
    v_r = v_d.rearrange("(n p) d -> p n d", p=P)
    o_r = o_d.rearrange("(n p) d -> p n d", p=P)

    from contextlib import ExitStack

    with tile.TileContext(nc) as tc, ExitStack() as ctx:
        ek = ctx.enter_context
        sing = ek(tc.tile_pool(name="sing", bufs=1))
        pnat = ek(tc.tile_pool(name="nat", bufs=1))     # rotating loads
        pstat = ek(tc.tile_pool(name="stat", bufs=1))   # bns/M2/rstd/tmp
        pbig = ek(tc.tile_pool(name="big", bufs=1))     # persistent tensors

        # --- constants ---
        ident = sing.tile([P, P], BF16)
        make_identity(nc, ident)
        dw_t = sing.tile([P, 1], F32)
        nc.sync.dma_start(out=dw_t, in_=dw_d[:, :].to_broadcast([P, 1]))
        dp_t = sing.tile([P, 1], F32)
        nc.sync.dma_start(out=dp_t, in_=dp_d[:, :].to_broadcast([P, 1]))
        dwq = sing.tile([P, 1], F32)   # dw * sqrt(7): tanh scale
        nc.vector.tensor_scalar_mul(dwq, dw_t, SQ7)
        dp2 = sing.tile([P, 1], F32)
        nc.vector.tensor_mul(dp2, dp_t, dp_t)
        dp4u = sing.tile([P, 1], F32)  # dyn^4 * UNB_S  (ln scale)
        nc.vector.tensor_mul(dp4u, dp2, dp2)
        nc.vector.tensor_scalar_mul(dp4u, dp4u, float(UNB_S))
        c2 = sing.tile([P, 1], F32)    # QSF * SCALE * dyn^2
        nc.vector.tensor_scalar_mul(c2, dp2, float(QSF * SCALE))
        eps_t = sing.tile([P, 1], F32)
        nc.vector.memset(eps_t, EPS)

        # stationary selectors
        ones8g = sing.tile([P, HG, H], BF16)   # ssq col-sum per pair
        nc.vector.memset(ones8g, 0.0)
        for g in range(HG):
            for j in range(2):
                nc.vector.memset(ones8g[64 * j : 64 * (j + 1), g,
                                        2 * g + j : 2 * g + j + 1], 1.0)
        # selm (m row -> 64-block broadcast) = ones8g^T, built via PE transpose
        # (single-partition memsets at bases 1,2,... are illegal)
        selm = sing.tile([H, HG, P], BF16)
        with tc.tile_pool(name="selm_ps", bufs=1, space="PSUM") as psel:
            pselm = psel.tile([H, HG, P], BF16, tag="pselm",
                              padded_shape=[H, HG, 512])
            for g in range(HG):
                nc.tensor.transpose(pselm[:, g, :], ones8g[:, g, :], ident)
            nc.vector.tensor_copy(selm, pselm)

        # --- persistent tensors ---
        tkb = pbig.tile([P, HG, S], BF16, tag="tkb")
        tqb = pbig.tile([P, HG, LC], BF16, tag="tqb")
        qsb = pbig.tile([P, HG, LC], BF16, tag="qsb")
        va = pbig.tile([P, NST, H, 66], BF16, tag="va")
        osb = pbig.tile([P, NLT, D], F32, tag="osb")
        msb = pbig.tile([H, LC], BF16, tag="msb")
        x8 = pbig.tile([H, LC], F32, tag="x8")
        gsb = pbig.tile([64, H, 65], BF16, tag="gsb")
        gd2 = pbig.tile([P, HG, P], BF16, tag="gd2")
        ks8g = pbig.tile([P, HG, H], BF16, tag="ks8g")
        nc.vector.memset(gd2, 0.0)
        nc.vector.memset(ks8g, 0.0)

        ptg = ek(tc.tile_pool(name="tg", bufs=1))

        v_queue = list(range(NST))

        with tc.tile_pool(name="pre_ps", bufs=1, space="PSUM") as ppre:
            G = ppre.tile([64, H, 65], F32, tag="G", padded_shape=[64, H, 96])

            def process_v(vi, eng):
                vn = pnat.tile([P, D], F32, tag="nat", name=f"vn{vi}", bufs=6)
                nc.sync.dma_start(out=vn, in_=v_r[:, vi, :])
                dst = va[:, vi, :, 0:E]
                src = vn.rearrange("p (h e) -> p h e", h=H)
                if eng is nc.scalar:
                    eng.copy(dst, src)
                else:
                    eng.tensor_copy(dst, src)

            def process_chunk(kind, i, src):
                nat = pnat.tile([P, D], F32, tag="nat", name=f"nat_{kind}{i}",
                                bufs=6)
                nc.sync.dma_start(out=nat, in_=src[:, i, :])
                # stats over H: M2 = ssq - ssum^2/H  (= sum (x-mu)^2)
                sq = pstat.tile([P, D], F32, tag="sq", bufs=3)
                nc.scalar.activation(sq, nat, AF.Square, bias=0.0, scale=1.0)
                red = pstat.tile([P, 2, E], F32, tag="red", bufs=4)
                nc.vector.tensor_reduce(
                    red[:, 0, :], nat.rearrange("p (h e) -> p e h", h=H),
                    axis=mybir.AxisListType.X, op=ALU.add)
                nc.vector.tensor_reduce(
                    red[:, 1, :], sq.rearrange("p (h e) -> p e h", h=H),
                    axis=mybir.AxisListType.X, op=ALU.add)
                m2 = pstat.tile([P, E], F32, tag="m2", bufs=4)
                nc.gpsimd.tensor_mul(m2, red[:, 0, :], red[:, 0, :])
                nc.vector.scalar_tensor_tensor(m2, m2, -1.0 / H, red[:, 1, :],
                                               op0=ALU.mult, op1=ALU.add)
                # rstd' = rsqrt(M2): magic seed + 1 Newton step (DVE)
                y = pstat.tile([P, E], F32, tag="y", bufs=4)
                nc.vector.tensor_scalar(out=y.bitcast(I32),
                                        in0=m2.bitcast(I32),
                                        scalar1=1, scalar2=None,
                                        op0=ALU.logical_shift_right)
                nc.vector.tensor_scalar(out=y.bitcast(I32),
                                        in0=y.bitcast(I32),
                                        scalar1=-1, scalar2=MAGIC,
                                        op0=ALU.mult, op1=ALU.add)
                a = pstat.tile([P, E], F32, tag="nra", bufs=2)
                nc.vector.tensor_mul(a, y, y)
                nc.vector.scalar_tensor_tensor(a, a, -0.5, m2,
                                               op0=ALU.mult, op1=ALU.mult)
                nc.vector.scalar_tensor_tensor(y, a, 1.5, y,
                                               op0=ALU.add, op1=ALU.mult)
                # tmp = nat * rstd' (broadcast over heads); tanh natural
                tmp = pstat.tile([P, D], F32, tag="tmp", bufs=3)
                nc.gpsimd.tensor_mul(tmp, nat, _hbcast(y))
                if kind == "k":
                    tg = ptg.tile([P, H, E], BF16, tag=f"tgk{i}",
                                  name=f"tgk{i}", bufs=1)
                else:
                    tg = ptg.tile([P, H, E], BF16, tag="tgq",
                                  name=f"tgq{i}", bufs=3)
                nc.scalar.activation(tg,
                                     tmp.rearrange("p (h e) -> p h e", h=H),
                                     AF.Tanh, bias=0.0, scale=dwq)
                # transpose into pair layout
                pt = ppre.tile([P, HG, P], BF16, tag="pt",
                               padded_shape=[P, HG, 512], bufs=3)
                for g in range(HG):
                    nc.tensor.transpose(
                        pt[:, g, :], tg[:, 2 * g : 2 * g + 2, :], ident)
                dst = (tkb if kind == "k" else tqb)[:, :, P * i : P * (i + 1)]
                if i % 2 == 0:
                    nc.vector.tensor_copy(dst, pt)
                else:
                    nc.scalar.copy(dst, pt)
                return tg

            v_engs = [nc.vector, nc.scalar, nc.gpsimd]
            tgk = []
            for i in range(NST):
                tgk.append(process_chunk("k", i, k_r))
                if i % 2 == 1 and v_queue:
                    vi = v_queue.pop(0)
                    process_v(vi, v_engs[vi % 3])
            # Gram per head: G[e, e'] + ksum col 64. Chains per head are
            # sequential (concurrent chains in one PSUM bank corrupt each
            # other via the start-flag zero-region).
            ones1 = sing.tile([P, 1], BF16)
            nc.vector.memset(ones1, 1.0)
            for h in range(H):
                for i in range(NST):
                    nc.tensor.matmul(G[:, h, 0:E], tgk[i][:, h, :],
                                     tgk[i][:, h, :],
                                     start=(i == 0), stop=(i == NST - 1))
                for i in range(NST):
                    nc.tensor.matmul(G[:, h, E : E + 1], tgk[i][:, h, :],
                                     ones1, start=(i == 0),
                                     stop=(i == NST - 1))
            for i in range(NLT):
                process_chunk("q", i, q_r)
                if v_queue:
                    vi = v_queue.pop(0)
                    process_v(vi, v_engs[vi % 3])
            while v_queue:
                vi = v_queue.pop(0)
                process_v(vi, v_engs[vi % 3])
            # ones column for the A@V denominator
            nc.vector.memset(
                va.rearrange("p n h c -> p (n h) c")[:, :, E : E + 1], 1.0)

            # G -> SBUF (scaled 1/S); build block-diag gd2 and ksum selectors
            nc.vector.tensor_scalar_mul(gsb, G, 1.0 / S)
            for g in range(HG):
                nc.vector.tensor_copy(gd2[0:64, g, 0:64], gsb[:, 2 * g, 0:64])
                nc.sync.dma_start(out=gd2[64:128, g, 64:128],
                                  in_=gsb[:, 2 * g + 1, 0:64])
                nc.vector.tensor_copy(ks8g[0:64, g, 2 * g : 2 * g + 1],
                                      gsb[:, 2 * g, 64:65])
                nc.sync.dma_start(out=ks8g[64:128, g, 2 * g + 1 : 2 * g + 2],
                                  in_=gsb[:, 2 * g + 1, 64:65])
            ln_c2 = sing.tile([H, 1], F32)
            nc.scalar.activation(ln_c2, c2[0:H, :], AF.Ln, bias=0.0, scale=1.0)

        # ============ score row stats -> m ============
        with tc.tile_pool(name="st_ps", bufs=1, space="PSUM") as pst:
            ssq_ps = pst.tile([H, LC], F32, tag="ssq")
            rs_ps = pst.tile([H, LC], F32, tag="rs")
            for g in range(HG):
                ws = pst.tile([P, LC], F32, tag="ws", bufs=2, name=f"ws{g}")
                for n0 in range(0, LC, 512):
                    nc.tensor.matmul(ws[:, n0 : n0 + 512], gd2[:, g, :],
                                     tqb[:, g, n0 : n0 + 512],
                                     start=True, stop=True)
                prod = pstat.tile([P, LC], BF16, tag="prod", bufs=2,
                                  name=f"prod{g}")
                nc.vector.tensor_mul(prod, ws, tqb[:, g, :])
                for n0 in range(0, LC, 512):
                    nc.tensor.matmul(ssq_ps[:, n0 : n0 + 512],
                                     ones8g[:, g, :], prod[:, n0 : n0 + 512],
                                     start=(g == 0), stop=(g == HG - 1))
                    nc.tensor.matmul(rs_ps[:, n0 : n0 + 512],
                                     ks8g[:, g, :], tqb[:, g, n0 : n0 + 512],
                                     start=(g == 0), stop=(g == HG - 1))
            # m = c2 / tau', tau'^2 = dyn^4*UNB*(ssq - rs^2) + eps
            nc.scalar.activation(x8, rs_ps, AF.Square, bias=0.0, scale=1.0)
            nc.vector.tensor_sub(x8, ssq_ps, x8)
            nc.scalar.activation(x8, x8, AF.Ln, bias=eps_t[0:H, :],
                                 scale=dp4u[0:H, :])
            nc.scalar.activation(msb, x8, AF.Exp, bias=ln_c2, scale=-0.5)

        # qs = tq * m (broadcast m rows across 64-partition blocks via PE)
        with tc.tile_pool(name="mb_ps", bufs=1, space="PSUM") as pmb:
            for g in range(HG):
                mb = pmb.tile([P, LC], F32, tag="mb", bufs=2, name=f"mb{g}")
                for n0 in range(0, LC, 512):
                    nc.tensor.matmul(mb[:, n0 : n0 + 512], selm[:, g, :],
                                     msb[:, n0 : n0 + 512],
                                     start=True, stop=True)
                nc.vector.tensor_mul(qsb[:, g, :], tqb[:, g, :], mb)

        # ============ main loop ============
        with tc.tile_pool(name="mm_ps", bufs=1, space="PSUM") as pmm, \
             tc.tile_pool(name="at_pool", bufs=1) as pat:

            def emit_exp(at, st_ps, kk):
                if kk >= NST - N_DVE_EXP:
                    nc.vector.tensor_scalar(
                        out=at[:, kk, :].bitcast(I16), in0=st_ps,
                        scalar1=SCH_A, scalar2=SCH_B,
                        op0=ALU.mult, op1=ALU.add)
                else:
                    nc.scalar.activation(at[:, kk, :], st_ps, AF.Exp,
                                         bias=0.0, scale=1.0 / QSF)

            def emit_av_lt(h, at, lt, po):
                for kx in range(NST):
                    nc.tensor.matmul(
                        po[:, lt, 0 : E + 1],
                        at[:, kx, lt * P : (lt + 1) * P],
                        va[:, kx, h, 0 : E + 1],
                        start=(kx == 0), stop=(kx == NST - 1))

            def emit_epilogue(h, po):
                rc = pstat.tile([P, NLT, 1], F32, tag="rc", bufs=2,
                                name=f"rc{h}")
                nc.vector.reciprocal(rc, po[:, :, E : E + 1])
                for lt in range(NLT):
                    nc.vector.tensor_scalar_mul(
                        osb[:, lt, E * h : E * (h + 1)], po[:, lt, 0:E],
                        rc[:, lt, :])
                    if h == H - 1:
                        nc.sync.dma_start(out=o_r[:, lt, :], in_=osb[:, lt, :])

            prev = None
            for h in range(H):
                g, hi = h // 2, h % 2
                tks = tkb[64 * hi : 64 * (hi + 1), g, :]
                qss = qsb[64 * hi : 64 * (hi + 1), g, :]
                at = pat.tile([P, NST, LC], BF16, tag="at", bufs=2,
                              name=f"at{h}")
                po_h = pmm.tile([P, NLT, E + 1], F32, tag="po", bufs=2,
                                name=f"po{h}", padded_shape=[P, NLT, P])
                for kk in range(NST):
                    st_ps = pmm.tile([P, LC], F32, tag="stp", bufs=2,
                                     name=f"st{h}_{kk}")
                    for n0 in range(0, LC, 512):
                        nc.tensor.matmul(
                            st_ps[:, n0 : n0 + 512],
                            tks[:, P * kk : P * (kk + 1)],
                            qss[:, n0 : n0 + 512],
                            start=True, stop=True)
                    emit_exp(at, st_ps, kk)
                    if prev is not None and kk % 2 == 1:
                        ph, pat_t, ppo = prev
                        emit_av_lt(ph, pat_t, kk // 2, ppo)
                        if kk == NST - 1:
                            emit_epilogue(ph, ppo)
                prev = (h, at, po_h)
            ph, pat_t, ppo = prev
            for lt in range(NLT):
                emit_av_lt(ph, pat_t, lt, ppo)
            emit_epilogue(ph, ppo)

    return nc


_nc_cache = None


def kernel(queries, keys, values, attn_mask=None, directional_weights=None,
           dynamic_param=None, **_unused):
    global _nc_cache, _last_exec_time_ns
    q = np.asarray(queries, dtype=np.float32)
    k = np.asarray(keys, dtype=np.float32)
    v = np.asarray(values, dtype=np.float32)
    if directional_weights is None:
        dw = np.ones((1, 1), dtype=np.float32)
    else:
        dw = np.asarray(directional_weights, dtype=np.float32).reshape(1, 1)
    if dynamic_param is None:
        dp = np.ones((1, 1), dtype=np.float32)
    else:
        dp = np.asarray(dynamic_param, dtype=np.float32).reshape(1, 1)

    if _nc_cache is None:
        nc = build_nc()
        nc.finalize()
        _nc_cache = nc
    nc = _nc_cache

    in_maps = []
    for c in range(8):
        b, lh = c // 2, c % 2
        in_maps.append({
            "q": np.ascontiguousarray(q[b, lh * LC : (lh + 1) * LC]).reshape(LC, D),
            "k": np.ascontiguousarray(k[b]).reshape(S, D),
            "v": np.ascontiguousarray(v[b]).reshape(S, D),
            "dw": dw, "dp": dp,
        })

    tracing = bool(os.environ.get("BASS_TRACE"))
    if tracing:
        _ensure_axon_hooks()
        import concourse.bass_utils as _bu

        _orig_upload = _bu.upload_artifacts
        _bu.upload_artifacts = lambda d: d
        try:
            res = run_bass_kernel_spmd(nc, in_maps, core_ids=list(range(8)))
        except Exception as e:
            print(f"traced run failed ({e!r}); retrying untraced", file=sys.stderr)
            os.environ["BASS_NEVER_TRACE"] = "1"
            try:
                res = run_bass_kernel_spmd(nc, in_maps, core_ids=list(range(8)))
            finally:
                os.environ.pop("BASS_NEVER_TRACE", None)
        finally:
            _bu.upload_artifacts = _orig_upload
    else:
        res = run_bass_kernel_spmd(nc, in_maps, core_ids=list(range(8)))
    _last_exec_time_ns = res.exec_time_ns

    out = np.empty((B, L, H, E), dtype=np.float32)
    for c in range(8):
        b, lh = c // 2, c % 2
        out[b, lh * LC : (lh + 1) * LC] = res.results[c]["o"].reshape(LC, H, E)
    return out
